# revision 1
# baseline (speedup 1.0000x reference)
"""CARC attention processor kernel for 8 Trainium2 NeuronCores.

Reference computation (B=4, L=1024, D=1280, H=20, DH=64, ALPHA=0.48):
    q/k/v = hidden @ Wq/Wk/Wv               -> per-head [B*H, L, 64]
    k,v   = concat([k, a*K_bg], [v, a*V_bg]) along kv-seq (kt = 2L = 2048)
    out   = softmax(q k^T / 8) v  -> merge heads -> @ Wo + bo

Sharding: core = 2*b + half owns batch b and heads [half*10, half*10+10).
Each core computes its 10 (batch, head) pairs end-to-end and a partial
out^T = (context_heads^T) @ Wo_rows; host sums the two half-partials per
batch. ALPHA is folded into K_bg/V_bg on the host; all matmul operands
are bf16 (fp32 PSUM accumulation), softmax runs in fp32 on ACT.

On-chip layout (per core):
    hT    [10][128,1024]  hidden[b]^T, D-major chunks (bf16)
    wq/wk [10][128,640]   weight D-chunks; lhsT slices give q^T/k^T directly
    qT/kT [5][128,1024]   head-dim on partitions (2 heads per tile)
    va    [8][128,10,65]  v tokens on partitions + ones column (Z trick)
    scores^T [kt=128,qt]  PSUM; exp -> E (bf16) -> PV with lhsT = v_aug
    ctx^T [65,512] PSUM   row 64 = softmax denominator Z
"""

import sys

if "/opt/trn_rl_repo" not in sys.path:
    sys.path.insert(0, "/opt/trn_rl_repo")

import numpy as np
import ml_dtypes

import concourse.bass as bass
import concourse.mybir as mybir
import concourse.tile as tile
from concourse import bacc
from concourse import bass_utils

B, L, D, H, DH = 4, 1024, 1280, 20, 64
ALPHA = 0.48
SCALE = 0.125  # 1/sqrt(DH)
HPC = 10       # heads per core
N_CORES = 8
BF16 = mybir.dt.bfloat16
F32 = mybir.dt.float32
nbf16 = ml_dtypes.bfloat16


def build_kernel(reps: int = 1):
    """Build + compile the per-core Bass module. reps>1 wraps the whole body
    in a hardware loop (used only for wall-clock timing in test harnesses)."""
    nc = bacc.Bacc("TRN2", target_bir_lowering=False, debug=False,
                   num_devices=N_CORES)

    hT_d = nc.dram_tensor("hT", [10, 128, 1024], BF16, kind="ExternalInput")
    wq_d = nc.dram_tensor("wq", [10, 128, 640], BF16, kind="ExternalInput")
    wk_d = nc.dram_tensor("wk", [10, 128, 640], BF16, kind="ExternalInput")
    wv_d = nc.dram_tensor("wv", [10, 128, 640], BF16, kind="ExternalInput")
    wo_d = nc.dram_tensor("wo", [5, 128, 1280], BF16, kind="ExternalInput")
    kbgT_d = nc.dram_tensor("kbgT", [5, 128, 1024], BF16, kind="ExternalInput")
    vbg_d = nc.dram_tensor("vbg", [8, 128, 10, 64], BF16, kind="ExternalInput")
    outT_d = nc.dram_tensor("outT", [1280, 1024], F32, kind="ExternalOutput")
    # scratch for the softmax-denominator reciprocal round trip
    zd_d = nc.dram_tensor("zd", [5, 2, 2, 512], F32, kind="Internal")
    rd_d = nc.dram_tensor("rd", [5, 2, 2, 512], F32, kind="Internal")

    with tile.TileContext(nc) as tc:
        from contextlib import ExitStack
        with ExitStack() as ctx:
            const = ctx.enter_context(tc.tile_pool(name="const", bufs=1))
            work = ctx.enter_context(tc.tile_pool(name="work", bufs=1))
            psum = ctx.enter_context(tc.tile_pool(name="psum", bufs=1, space="PSUM"))

            def body(_it=None):
                Exp = mybir.ActivationFunctionType.Exp

                # ---- persistent SBUF tiles + input DMAs ----
                hT = [const.tile([128, 1024], BF16, name=f"hT{i}", tag=f"hT{i}")
                      for i in range(10)]
                for i in range(10):
                    nc.sync.dma_start(out=hT[i], in_=hT_d.ap()[i])
                wq = [const.tile([128, 640], BF16, name=f"wq{i}", tag=f"wq{i}")
                      for i in range(10)]
                wk = [const.tile([128, 640], BF16, name=f"wk{i}", tag=f"wk{i}")
                      for i in range(10)]
                for i in range(10):
                    nc.sync.dma_start(out=wq[i], in_=wq_d.ap()[i])
                    nc.sync.dma_start(out=wk[i], in_=wk_d.ap()[i])
                kbgT = [const.tile([128, 1024], BF16, name=f"kbgT{t}", tag=f"kbgT{t}")
                        for t in range(5)]
                for t in range(5):
                    nc.sync.dma_start(out=kbgT[t], in_=kbgT_d.ap()[t])
                wv = [const.tile([128, 640], BF16, name=f"wv{i}", tag=f"wv{i}")
                      for i in range(10)]
                for i in range(10):
                    nc.sync.dma_start(out=wv[i], in_=wv_d.ap()[i])
                va = [const.tile([128, 10, 65], BF16, name=f"va{t}", tag=f"va{t}")
                      for t in range(8)]
                vb = [const.tile([128, 10, 65], BF16, name=f"vb{t}", tag=f"vb{t}")
                      for t in range(8)]
                for t in range(8):
                    nc.sync.dma_start(out=vb[t][:, :, 0:64], in_=vbg_d.ap()[t])
                    nc.vector.memset(vb[t][:, :, 64:65], 1.0)
                    nc.vector.memset(va[t][:, :, 64:65], 1.0)
                wo = [const.tile([128, 1280], BF16, name=f"wo{j}", tag=f"wo{j}")
                      for j in range(5)]
                for j in range(5):
                    nc.sync.dma_start(out=wo[j], in_=wo_d.ap()[j])

                qT = [const.tile([128, 1024], BF16, name=f"qT{g}", tag=f"qT{g}")
                      for g in range(5)]
                kT = [const.tile([128, 1024], BF16, name=f"kT{g}", tag=f"kT{g}")
                      for g in range(5)]
                ctxn = [const.tile([128, 1024], BF16, name=f"ctxn{g}", tag=f"ctxn{g}")
                        for g in range(5)]

                for g in range(5):
                    gs = bass.ts(g, 128)  # head-pair column slice

                    # ---- q^T / k^T projections for heads (2g, 2g+1) ----
                    for (w_sb, dst) in ((wq, qT), (wk, kT)):
                        for half in range(2):
                            qs = bass.ts(half, 512)
                            ps = psum.tile([128, 512], F32, name=f"pp{g}{half}",
                                           tag="ps_s", bufs=2)
                            for i in range(10):
                                nc.tensor.matmul(ps, w_sb[i][:, gs], hT[i][:, qs],
                                                 start=(i == 0), stop=(i == 9))
                            nc.vector.tensor_copy(out=dst[g][:, qs], in_=ps)

                    # ---- v projection for heads (2g, 2g+1) ----
                    for t in range(8):
                        ts_ = bass.ts(t, 128)
                        ps = psum.tile([128, 128], F32, name=f"pv{g}{t}",
                                       tag="ps_s", bufs=2)
                        for i in range(10):
                            nc.tensor.matmul(ps, hT[i][:, ts_], wv[i][:, gs],
                                             start=(i == 0), stop=(i == 9))
                        nc.vector.tensor_copy(
                            out=va[t][:, 2 * g:2 * g + 2, 0:64],
                            in_=ps.rearrange("p (a b) -> p a b", a=2))

                    # ---- attention for pair p in {2g, 2g+1} ----
                    cps = [[psum.tile([65, 512], F32, name=f"cx{g}{p}{h}",
                                      tag="ps_c", bufs=4)
                            for h in range(2)] for p in range(2)]
                    for c in range(16):
                        if c < 8:
                            kt_src = kT[g][:, bass.ts(c, 128)]
                            v_of = lambda p, _c=c: va[_c][:, 2 * g + p, :]
                        else:
                            kt_src = kbgT[g][:, bass.ts(c - 8, 128)]
                            v_of = lambda p, _c=c - 8: vb[_c][:, 2 * g + p, :]
                        es = []
                        for p in range(2):
                            rows = slice(p * 64, p * 64 + 64)
                            sc = psum.tile([128, 1024], F32, name=f"sc{g}{c}{p}",
                                           tag="ps_s", bufs=2)
                            for half in range(2):
                                nc.tensor.matmul(
                                    sc[:, bass.ts(half, 512)],
                                    kt_src[rows, :],
                                    qT[g][rows, bass.ts(half, 512)],
                                    start=True, stop=True,
                                    tile_position=(p * 64, 0))
                            e = work.tile([128, 1024], BF16, name=f"e{g}{c}{p}",
                                          tag="e", bufs=4)
                            nc.scalar.activation(out=e, in_=sc, func=Exp,
                                                 scale=SCALE)
                            es.append(e)
                        for p in range(2):
                            for half in range(2):
                                nc.tensor.matmul(
                                    cps[p][half], v_of(p),
                                    es[p][:, bass.ts(half, 512)],
                                    start=(c == 0), stop=(c == 15))

                    # ---- normalize context, stage as bf16 ctx^T ----
                    for p in range(2):
                        rows = slice(p * 64, p * 64 + 64)
                        for half in range(2):
                            qs = bass.ts(half, 512)
                            cs = work.tile([65, 512], F32, name=f"cs{g}{p}{half}",
                                           tag="cs", bufs=4)
                            nc.vector.tensor_copy(out=cs, in_=cps[p][half])
                            zdst = zd_d.ap()[g, p, half].rearrange(
                                "(a b) -> a b", a=1)
                            nc.sync.dma_start(out=zdst, in_=cs[64:65, :])
                            zp = work.tile([128, 4], F32, name=f"zp{g}{p}{half}",
                                           tag="zp", bufs=4)
                            nc.sync.dma_start(
                                out=zp,
                                in_=zd_d.ap()[g, p, half].rearrange(
                                    "(a b) -> a b", a=128))
                            rp = work.tile([128, 4], F32, name=f"rp{g}{p}{half}",
                                           tag="rp", bufs=4)
                            nc.vector.reciprocal(rp, zp)
                            nc.sync.dma_start(
                                out=rd_d.ap()[g, p, half].rearrange(
                                    "(a b) -> a b", a=128),
                                in_=rp)
                            rflat = rd_d.ap()[g, p, half]
                            rb = work.tile([64, 512], F32, name=f"rb{g}{p}{half}",
                                           tag="rb", bufs=4)
                            nc.gpsimd.dma_start(
                                out=rb,
                                in_=bass.AP(tensor=rflat.tensor,
                                            offset=rflat.offset,
                                            ap=[[0, 64]] + list(rflat.ap)))
                            nc.vector.tensor_mul(out=ctxn[g][rows, qs],
                                                 in0=cs[0:64, :], in1=rb)

                # ---- output projection: out^T = Wo_rows^T @ ctx^T ----
                for dt_ in range(10):
                    ds_ = bass.ts(dt_, 128)
                    for t in range(2):
                        ts_ = bass.ts(t, 512)
                        ps = psum.tile([128, 512], F32, name=f"po{dt_}{t}",
                                       tag="ps_c", bufs=4)
                        for j in range(5):
                            nc.tensor.matmul(ps, wo[j][:, ds_], ctxn[j][:, ts_],
                                             start=(j == 0), stop=(j == 4))
                        osb = work.tile([128, 512], F32, name=f"o{dt_}{t}",
                                        tag="osb", bufs=4)
                        nc.vector.tensor_copy(out=osb, in_=ps)
                        nc.sync.dma_start(
                            out=outT_d.ap()[dt_ * 128:(dt_ + 1) * 128, ts_],
                            in_=osb)

            if reps == 1:
                body()
            else:
                with tc.For_i(0, reps, 1) as it:
                    body(it)

    nc.compile()
    return nc


def shard_inputs(inputs):
    """Full fp32 inputs -> 8 per-core input maps (host-side cast/layout)."""
    hs = np.asarray(inputs["hidden_states"], dtype=np.float32)
    Wq = np.asarray(inputs["Wq"], dtype=np.float32)
    Wk = np.asarray(inputs["Wk"], dtype=np.float32)
    Wv = np.asarray(inputs["Wv"], dtype=np.float32)
    Wo = np.asarray(inputs["Wo"], dtype=np.float32)
    K_bg = np.asarray(inputs["K_bg"], dtype=np.float32)
    V_bg = np.asarray(inputs["V_bg"], dtype=np.float32)

    in_maps = []
    for c in range(N_CORES):
        b, half = divmod(c, 2)
        col0 = half * HPC * DH  # 0 or 640
        bh0 = b * H + half * HPC
        hT = np.ascontiguousarray(hs[b].T).astype(nbf16).reshape(10, 128, 1024)
        wq = np.ascontiguousarray(Wq[:, col0:col0 + 640]).astype(nbf16)
        wk = np.ascontiguousarray(Wk[:, col0:col0 + 640]).astype(nbf16)
        wv = np.ascontiguousarray(Wv[:, col0:col0 + 640]).astype(nbf16)
        wo = np.ascontiguousarray(Wo[col0:col0 + 640, :]).astype(nbf16)
        kbgT = np.ascontiguousarray(
            (ALPHA * K_bg[bh0:bh0 + 10]).transpose(0, 2, 1)).astype(nbf16)
        vbg = np.ascontiguousarray(
            (ALPHA * V_bg[bh0:bh0 + 10]).reshape(10, 8, 128, 64)
            .transpose(1, 2, 0, 3)).astype(nbf16)
        in_maps.append({
            "hT": hT,
            "wq": wq.reshape(10, 128, 640),
            "wk": wk.reshape(10, 128, 640),
            "wv": wv.reshape(10, 128, 640),
            "wo": wo.reshape(5, 128, 1280),
            "kbgT": kbgT.reshape(5, 128, 1024),
            "vbg": vbg,
        })
    return in_maps


_NC_CACHE = {}


def _get_nc(reps=1):
    if reps not in _NC_CACHE:
        _NC_CACHE[reps] = build_kernel(reps)
    return _NC_CACHE[reps]


def kernel(**inputs):
    nc = _get_nc(1)
    in_maps = shard_inputs(inputs)
    res = bass_utils.run_bass_kernel_spmd(nc, in_maps,
                                          core_ids=list(range(N_CORES)))
    outs = np.stack([res.results[c]["outT"] for c in range(N_CORES)])
    out = outs.reshape(B, 2, D, L).sum(axis=1).transpose(0, 2, 1)
    out = out + np.asarray(inputs["bo"], dtype=np.float32)
    return np.ascontiguousarray(out, dtype=np.float32)


# revision 2
# speedup vs baseline: 1.3146x; 1.3146x over previous
"""CARC attention processor kernel for 8 Trainium2 NeuronCores.

Reference computation (B=4, L=1024, D=1280, H=20, DH=64, ALPHA=0.48):
    q/k/v = hidden @ Wq/Wk/Wv               -> per-head [B*H, L, 64]
    k,v   = concat([k, a*K_bg], [v, a*V_bg]) along kv-seq (kt = 2L = 2048)
    out   = softmax(q k^T / 8) v  -> merge heads -> @ Wo + bo

Sharding: core = 2*b + half owns batch b and heads [half*10, half*10+10).
Each core computes its 10 (batch, head) pairs end-to-end and a partial
out^T = (context_heads^T) @ Wo_rows; host sums the two half-partials per
batch. ALPHA is folded into K_bg/V_bg on the host; all matmul operands
are bf16 (fp32 PSUM accumulation), softmax runs in fp32 on ACT.

Pipeline structure (per head-pair group g = heads 2g, 2g+1):
  - scores^T [kt=128, qt] per 128-chunk of kt, computed as a row-packed
    matmul pair (tile_position rows 0-63 / 64-127 = head A / head B) into
    one [128, 1024] PSUM tile laid out [A | B]; a single FD=1024 exp on
    ACT produces E (bf16).  PV accumulates ctx^T [65, qt] with a
    ones-augmented V (row 64 = softmax denominator Z).
  - The kv loop visits background chunks first so self-V projections
    (interleaved as PE filler) are ready when the self chunks arrive.
  - q/k projections for group g+1 are interleaved into group g's
    attention as PE filler, keeping ACT (the exp spine) saturated.
  - Z reciprocal: Z row -> DRAM -> [128,4] -> reciprocal -> DRAM ->
    partition-broadcast [64,512]; ctx^T * (1/Z) -> bf16 ctxn.
  - out^T accumulated from Wo chunks at the end.
"""

import sys

if "/opt/trn_rl_repo" not in sys.path:
    sys.path.insert(0, "/opt/trn_rl_repo")

from collections import deque
from contextlib import ExitStack

import numpy as np
import ml_dtypes

import concourse.bass as bass
import concourse.mybir as mybir
import concourse.tile as tile
from concourse import bacc
from concourse import bass_utils

B, L, D, H, DH = 4, 1024, 1280, 20, 64
ALPHA = 0.48
SCALE = 0.125  # 1/sqrt(DH)
HPC = 10       # heads per core
N_CORES = 8
BF16 = mybir.dt.bfloat16
F32 = mybir.dt.float32
nbf16 = ml_dtypes.bfloat16


def build_kernel(reps: int = 1):
    """Build + compile the per-core Bass module. reps>1 wraps the whole body
    in a hardware loop (used only for wall-clock timing in test harnesses)."""
    nc = bacc.Bacc("TRN2", target_bir_lowering=False, debug=False,
                   num_devices=N_CORES)

    hT_d = nc.dram_tensor("hT", [10, 128, 1024], BF16, kind="ExternalInput")
    wq_d = nc.dram_tensor("wq", [10, 128, 640], BF16, kind="ExternalInput")
    wk_d = nc.dram_tensor("wk", [10, 128, 640], BF16, kind="ExternalInput")
    wv_d = nc.dram_tensor("wv", [10, 128, 640], BF16, kind="ExternalInput")
    wo_d = nc.dram_tensor("wo", [5, 128, 1280], BF16, kind="ExternalInput")
    kbgT_d = nc.dram_tensor("kbgT", [5, 128, 1024], BF16, kind="ExternalInput")
    vbg_d = nc.dram_tensor("vbg", [8, 128, 10, 64], BF16, kind="ExternalInput")
    outT_d = nc.dram_tensor("outT", [1280, 1024], F32, kind="ExternalOutput")
    # scratch for the softmax-denominator reciprocal round trip
    zd_d = nc.dram_tensor("zd", [5, 2, 2, 512], F32, kind="Internal")
    rd_d = nc.dram_tensor("rd", [5, 2, 2, 512], F32, kind="Internal")

    with tile.TileContext(nc) as tc:
        with ExitStack() as ctx:
            const = ctx.enter_context(tc.tile_pool(name="const", bufs=1))
            work = ctx.enter_context(tc.tile_pool(name="work", bufs=1))
            psum = ctx.enter_context(tc.tile_pool(name="psum", bufs=1, space="PSUM"))

            def body(_it=None):
                Exp = mybir.ActivationFunctionType.Exp

                # ---- persistent SBUF tiles + input DMAs ----
                hT = [const.tile([128, 1024], BF16, name=f"hT{i}", tag=f"hT{i}")
                      for i in range(10)]
                wq = [const.tile([128, 640], BF16, name=f"wq{i}", tag=f"wq{i}")
                      for i in range(10)]
                wk = [const.tile([128, 640], BF16, name=f"wk{i}", tag=f"wk{i}")
                      for i in range(10)]
                for i in range(10):
                    nc.sync.dma_start(out=hT[i], in_=hT_d.ap()[i])
                    nc.sync.dma_start(out=wq[i], in_=wq_d.ap()[i])
                    nc.sync.dma_start(out=wk[i], in_=wk_d.ap()[i])
                kbgT = [const.tile([128, 1024], BF16, name=f"kbgT{t}", tag=f"kbgT{t}")
                        for t in range(5)]
                for t in range(5):
                    nc.sync.dma_start(out=kbgT[t], in_=kbgT_d.ap()[t])
                wv = [const.tile([128, 640], BF16, name=f"wv{i}", tag=f"wv{i}")
                      for i in range(10)]
                for i in range(10):
                    nc.sync.dma_start(out=wv[i], in_=wv_d.ap()[i])
                va = [const.tile([128, 10, 65], BF16, name=f"va{t}", tag=f"va{t}")
                      for t in range(8)]
                vb = [const.tile([128, 10, 65], BF16, name=f"vb{t}", tag=f"vb{t}")
                      for t in range(8)]
                for t in range(8):
                    nc.sync.dma_start(out=vb[t][:, :, 0:64], in_=vbg_d.ap()[t])
                    nc.vector.memset(vb[t][:, :, 64:65], 1.0)
                    nc.vector.memset(va[t][:, :, 64:65], 1.0)
                wo = [const.tile([128, 1280], BF16, name=f"wo{j}", tag=f"wo{j}")
                      for j in range(5)]
                for j in range(5):
                    nc.sync.dma_start(out=wo[j], in_=wo_d.ap()[j])

                qT = [const.tile([128, 1024], BF16, name=f"qT{g}", tag=f"qT{g}")
                      for g in range(5)]
                kT = [const.tile([128, 1024], BF16, name=f"kT{g}", tag=f"kT{g}")
                      for g in range(5)]
                ctxn = [const.tile([128, 1024], BF16, name=f"ctxn{g}", tag=f"ctxn{g}")
                        for g in range(5)]

                # ---- projection emitters (used upfront for g=0, as PE
                # filler interleaved into attention for g>=1) ----
                def qk_group_steps(g, w_sb, dst, half, psum_tag):
                    """Two emission steps (5 matmuls each) accumulating one
                    [128, 512] q^T/k^T projection tile, then evacuate."""
                    gs = bass.ts(g, 128)
                    qs = bass.ts(half, 512)
                    state = {}

                    def step1():
                        state["ps"] = psum.tile(
                            [128, 512], F32, name=f"pp{g}{half}",
                            uniquify=True, tag=psum_tag,
                            bufs=2 if psum_tag == "sc" else 1)
                        for i in range(5):
                            nc.tensor.matmul(state["ps"], w_sb[i][:, gs],
                                             hT[i][:, qs],
                                             start=(i == 0), stop=False)

                    def step2():
                        for i in range(5, 10):
                            nc.tensor.matmul(state["ps"], w_sb[i][:, gs],
                                             hT[i][:, qs],
                                             start=False, stop=(i == 9))
                        nc.vector.tensor_copy(out=dst[g][:, qs], in_=state["ps"])

                    return [step1, step2]

                def v_group_step(g, t):
                    """One emission step: v projection for heads (2g, 2g+1),
                    token tile t -> va[t][:, 2g:2g+2, 0:64]."""
                    gs = bass.ts(g, 128)
                    ts_ = bass.ts(t, 128)

                    def step():
                        ps = psum.tile([128, 128], F32, name=f"pv{g}{t}",
                                       uniquify=True, tag="pp", bufs=1)
                        for i in range(10):
                            nc.tensor.matmul(ps, hT[i][:, ts_], wv[i][:, gs],
                                             start=(i == 0), stop=(i == 9))
                        nc.vector.tensor_copy(
                            out=va[t][:, 2 * g:2 * g + 2, 0:64],
                            in_=ps.rearrange("p (a b) -> p a b", a=2))

                    return step

                # g=0 q/k projections upfront (through the idle sc slots)
                for half in range(2):
                    for st in qk_group_steps(0, wq, qT, half, "sc"):
                        st()
                    for st in qk_group_steps(0, wk, kT, half, "sc"):
                        st()

                CORDER = list(range(8, 16)) + list(range(8))  # bg chunks first

                def emit_qk_mm(g, c, half, sc):
                    """Row-packed scores^T matmul pair for kv-chunk c into
                    sc = [128, 1024] PSUM laid out [A | B]."""
                    if c < 8:
                        kt_src = kT[g][:, bass.ts(c, 128)]
                    else:
                        kt_src = kbgT[g][:, bass.ts(c - 8, 128)]
                    for p in range(2):
                        rows = slice(p * 64, p * 64 + 64)
                        nc.tensor.matmul(
                            sc[:, bass.ts(p, 512)],
                            kt_src[rows, :],
                            qT[g][rows, bass.ts(half, 512)],
                            start=True, stop=True,
                            tile_position=(p * 64, 0))

                for g in range(5):
                    for half in range(2):
                        qs = bass.ts(half, 512)
                        # filler steps for this (g, half)
                        filler = deque()
                        if half == 0:
                            for t in range(8):
                                filler.append(v_group_step(g, t))
                        elif g < 4:
                            for (w_sb, dst) in ((wq, qT), (wk, kT)):
                                for h2 in range(2):
                                    filler.extend(qk_group_steps(
                                        g + 1, w_sb, dst, h2, "pp"))

                        cps = [psum.tile([65, 512], F32, name=f"cx{g}{half}{p}",
                                         tag="cx", bufs=3) for p in range(2)]
                        sc_tiles = {}
                        c0 = CORDER[0]
                        sc_tiles[c0] = psum.tile([128, 1024], F32,
                                                 name=f"sc{g}{half}{c0}",
                                                 uniquify=True, tag="sc", bufs=2)
                        emit_qk_mm(g, c0, half, sc_tiles[c0])
                        for k, c in enumerate(CORDER):
                            e = work.tile([128, 1024], BF16, name=f"e{g}{half}{c}",
                                          uniquify=True, tag="e", bufs=3)
                            nc.scalar.activation(out=e, in_=sc_tiles.pop(c),
                                                 func=Exp, scale=SCALE)
                            if k + 1 < 16:
                                cn = CORDER[k + 1]
                                sc_tiles[cn] = psum.tile(
                                    [128, 1024], F32, name=f"sc{g}{half}{cn}",
                                    uniquify=True, tag="sc", bufs=2)
                                emit_qk_mm(g, cn, half, sc_tiles[cn])
                            # PE filler while ACT computes exp
                            if half == 0:
                                if k < 8 and filler:
                                    filler.popleft()()
                            elif k % 2 == 0 and filler:
                                filler.popleft()()
                            for p in range(2):
                                v_src = (va if c < 8 else vb)[c % 8][:, 2 * g + p, :]
                                nc.tensor.matmul(
                                    cps[p], v_src, e[:, bass.ts(p, 512)],
                                    start=(k == 0), stop=(k == 15))
                        while filler:
                            filler.popleft()()

                        # ---- normalize context, stage as bf16 ctx^T ----
                        for p in range(2):
                            rows = slice(p * 64, p * 64 + 64)
                            cs = work.tile([65, 512], F32, name=f"cs{g}{half}{p}",
                                           uniquify=True, tag="cs", bufs=4)
                            nc.vector.tensor_copy(out=cs, in_=cps[p])
                            zdst = zd_d.ap()[g, p, half].rearrange(
                                "(a b) -> a b", a=1)
                            nc.sync.dma_start(out=zdst, in_=cs[64:65, :])
                            zp = work.tile([128, 4], F32, name=f"zp{g}{half}{p}",
                                           uniquify=True, tag="zp", bufs=4)
                            nc.sync.dma_start(
                                out=zp,
                                in_=zd_d.ap()[g, p, half].rearrange(
                                    "(a b) -> a b", a=128))
                            rp = work.tile([128, 4], F32, name=f"rp{g}{half}{p}",
                                           uniquify=True, tag="rp", bufs=4)
                            nc.vector.reciprocal(rp, zp)
                            nc.sync.dma_start(
                                out=rd_d.ap()[g, p, half].rearrange(
                                    "(a b) -> a b", a=128),
                                in_=rp)
                            rflat = rd_d.ap()[g, p, half]
                            rb = work.tile([64, 512], F32, name=f"rb{g}{half}{p}",
                                           uniquify=True, tag="rb", bufs=4)
                            nc.gpsimd.dma_start(
                                out=rb,
                                in_=bass.AP(tensor=rflat.tensor,
                                            offset=rflat.offset,
                                            ap=[[0, 64]] + list(rflat.ap)))
                            nc.vector.tensor_mul(out=ctxn[g][rows, qs],
                                                 in0=cs[0:64, :], in1=rb)

                # ---- output projection: out^T = Wo_rows^T @ ctx^T ----
                for dt_ in range(10):
                    ds_ = bass.ts(dt_, 128)
                    for t in range(2):
                        ts_ = bass.ts(t, 512)
                        ps = psum.tile([128, 512], F32, name=f"po{dt_}{t}",
                                       uniquify=True, tag="cx", bufs=3)
                        for j in range(5):
                            nc.tensor.matmul(ps, wo[j][:, ds_], ctxn[j][:, ts_],
                                             start=(j == 0), stop=(j == 4))
                        osb = work.tile([128, 512], F32, name=f"o{dt_}{t}",
                                        uniquify=True, tag="osb", bufs=4)
                        nc.vector.tensor_copy(out=osb, in_=ps)
                        nc.sync.dma_start(
                            out=outT_d.ap()[dt_ * 128:(dt_ + 1) * 128, ts_],
                            in_=osb)

            if reps == 1:
                body()
            else:
                with tc.For_i(0, reps, 1) as it:
                    body(it)

    nc.compile()
    return nc


def shard_inputs(inputs):
    """Full fp32 inputs -> 8 per-core input maps (host-side cast/layout)."""
    hs = np.asarray(inputs["hidden_states"], dtype=np.float32)
    Wq = np.asarray(inputs["Wq"], dtype=np.float32)
    Wk = np.asarray(inputs["Wk"], dtype=np.float32)
    Wv = np.asarray(inputs["Wv"], dtype=np.float32)
    Wo = np.asarray(inputs["Wo"], dtype=np.float32)
    K_bg = np.asarray(inputs["K_bg"], dtype=np.float32)
    V_bg = np.asarray(inputs["V_bg"], dtype=np.float32)

    in_maps = []
    for c in range(N_CORES):
        b, half = divmod(c, 2)
        col0 = half * HPC * DH  # 0 or 640
        bh0 = b * H + half * HPC
        hT = np.ascontiguousarray(hs[b].T).astype(nbf16).reshape(10, 128, 1024)
        wq = np.ascontiguousarray(Wq[:, col0:col0 + 640]).astype(nbf16)
        wk = np.ascontiguousarray(Wk[:, col0:col0 + 640]).astype(nbf16)
        wv = np.ascontiguousarray(Wv[:, col0:col0 + 640]).astype(nbf16)
        wo = np.ascontiguousarray(Wo[col0:col0 + 640, :]).astype(nbf16)
        kbgT = np.ascontiguousarray(
            (ALPHA * K_bg[bh0:bh0 + 10]).transpose(0, 2, 1)).astype(nbf16)
        vbg = np.ascontiguousarray(
            (ALPHA * V_bg[bh0:bh0 + 10]).reshape(10, 8, 128, 64)
            .transpose(1, 2, 0, 3)).astype(nbf16)
        in_maps.append({
            "hT": hT,
            "wq": wq.reshape(10, 128, 640),
            "wk": wk.reshape(10, 128, 640),
            "wv": wv.reshape(10, 128, 640),
            "wo": wo.reshape(5, 128, 1280),
            "kbgT": kbgT.reshape(5, 128, 1024),
            "vbg": vbg,
        })
    return in_maps


_NC_CACHE = {}


def _get_nc(reps=1):
    if reps not in _NC_CACHE:
        _NC_CACHE[reps] = build_kernel(reps)
    return _NC_CACHE[reps]


def kernel(**inputs):
    nc = _get_nc(1)
    in_maps = shard_inputs(inputs)
    res = bass_utils.run_bass_kernel_spmd(nc, in_maps,
                                          core_ids=list(range(N_CORES)))
    outs = np.stack([res.results[c]["outT"] for c in range(N_CORES)])
    out = outs.reshape(B, 2, D, L).sum(axis=1).transpose(0, 2, 1)
    out = out + np.asarray(inputs["bo"], dtype=np.float32)
    return np.ascontiguousarray(out, dtype=np.float32)


# revision 10
# speedup vs baseline: 1.3460x; 1.0239x over previous
"""CARC attention processor kernel for 8 Trainium2 NeuronCores.

Reference computation (B=4, L=1024, D=1280, H=20, DH=64, ALPHA=0.48):
    q/k/v = hidden @ Wq/Wk/Wv               -> per-head [B*H, L, 64]
    k,v   = concat([k, a*K_bg], [v, a*V_bg]) along kv-seq (kt = 2L = 2048)
    out   = softmax(q k^T / 8) v  -> merge heads -> @ Wo + bo

Sharding: core = 2*b + half owns batch b and heads [half*10, half*10+10).
Each core computes its 10 (batch, head) pairs end-to-end and a partial
out^T = (context_heads^T) @ Wo_rows; host sums the two half-partials per
batch. ALPHA is folded into K_bg/V_bg on the host; all matmul operands
are bf16 (fp32 PSUM accumulation), softmax runs in fp32 on ACT.

Pipeline structure (per head-pair group g = heads 2g, 2g+1):
  - scores^T [kt=128, qt] per 128-chunk of kt, computed as a row-packed
    matmul pair (tile_position rows 0-63 / 64-127 = head A / head B) into
    one [128, 1024] PSUM tile laid out [A | B]; a single FD=1024 exp on
    ACT produces E (bf16).  PV accumulates ctx^T [65, qt] with a
    ones-augmented V (row 64 = softmax denominator Z).
  - The kv loop visits background chunks first so self-V projections
    (interleaved as PE filler) are ready when the self chunks arrive.
  - q/k projections for group g+1 are interleaved into group g's
    attention as PE filler, keeping ACT (the exp spine) saturated.
  - Z reciprocal: Z row -> DRAM -> [128,4] -> reciprocal -> DRAM ->
    partition-broadcast [64,512]; ctx^T * (1/Z) -> bf16 ctxn.
  - out^T accumulated from Wo chunks at the end.
"""

import sys

if "/opt/trn_rl_repo" not in sys.path:
    sys.path.insert(0, "/opt/trn_rl_repo")

from collections import deque
from contextlib import ExitStack

import numpy as np
import ml_dtypes

import concourse.bass as bass
import concourse.mybir as mybir
import concourse.tile as tile
from concourse import bacc
from concourse import bass_utils

B, L, D, H, DH = 4, 1024, 1280, 20, 64
ALPHA = 0.48
SCALE = 0.125  # 1/sqrt(DH)
HPC = 10       # heads per core
N_CORES = 8
BF16 = mybir.dt.bfloat16
F32 = mybir.dt.float32
nbf16 = ml_dtypes.bfloat16


def build_kernel(reps: int = 1):
    """Build + compile the per-core Bass module. reps>1 wraps the whole body
    in a hardware loop (used only for wall-clock timing in test harnesses)."""
    nc = bacc.Bacc("TRN2", target_bir_lowering=False, debug=False,
                   num_devices=N_CORES)

    hT_d = nc.dram_tensor("hT", [10, 128, 1024], BF16, kind="ExternalInput")
    wq_d = nc.dram_tensor("wq", [10, 128, 640], BF16, kind="ExternalInput")
    wk_d = nc.dram_tensor("wk", [10, 128, 640], BF16, kind="ExternalInput")
    wv_d = nc.dram_tensor("wv", [10, 128, 640], BF16, kind="ExternalInput")
    wo_d = nc.dram_tensor("wo", [5, 128, 1280], BF16, kind="ExternalInput")
    kbgT_d = nc.dram_tensor("kbgT", [5, 128, 1024], BF16, kind="ExternalInput")
    vbg_d = nc.dram_tensor("vbg", [8, 128, 10, 64], BF16, kind="ExternalInput")
    outT_d = nc.dram_tensor("outT", [1280, 1024], F32, kind="ExternalOutput")
    # scratch for the softmax-denominator reciprocal round trip
    zd_d = nc.dram_tensor("zd", [5, 2, 2, 512], F32, kind="Internal")
    rd_d = nc.dram_tensor("rd", [5, 2, 2, 512], F32, kind="Internal")

    with tile.TileContext(nc) as tc:
        with ExitStack() as ctx:
            const = ctx.enter_context(tc.tile_pool(name="const", bufs=1))
            work = ctx.enter_context(tc.tile_pool(name="work", bufs=1))
            psum = ctx.enter_context(tc.tile_pool(name="psum", bufs=1, space="PSUM"))

            def body(_it=None):
                Exp = mybir.ActivationFunctionType.Exp

                # ---- persistent SBUF tiles + input DMAs ----
                hT = [const.tile([128, 1024], BF16, name=f"hT{i}", tag=f"hT{i}")
                      for i in range(10)]
                wq = [const.tile([128, 640], BF16, name=f"wq{i}", tag=f"wq{i}")
                      for i in range(10)]
                wk = [const.tile([128, 640], BF16, name=f"wk{i}", tag=f"wk{i}")
                      for i in range(10)]
                # Interleave the startup loads across the SP HWDGE ring and
                # the gpsimd SWDGE ring (both idle at startup; the ACT ring
                # would delay the first activations behind its triggers).
                rings = [nc.sync, nc.gpsimd]
                _n = 0

                def dma(out, in_):
                    nonlocal _n
                    rings[_n % 2].dma_start(out=out, in_=in_)
                    _n += 1

                kbgT = [const.tile([128, 1024], BF16, name=f"kbgT{t}", tag=f"kbgT{t}")
                        for t in range(5)]
                dma(kbgT[0], kbgT_d.ap()[0])
                for i in range(10):
                    dma(hT[i], hT_d.ap()[i])
                    dma(wq[i], wq_d.ap()[i])
                    dma(wk[i], wk_d.ap()[i])
                va = [const.tile([128, 10, 65], BF16, name=f"va{t}", tag=f"va{t}")
                      for t in range(8)]
                vb = [const.tile([128, 10, 65], BF16, name=f"vb{t}", tag=f"vb{t}")
                      for t in range(8)]
                wv = [const.tile([128, 640], BF16, name=f"wv{i}", tag=f"wv{i}")
                      for i in range(10)]
                for t in range(8):
                    dma(vb[t][:, :, 0:64], vbg_d.ap()[t])
                    nc.vector.memset(vb[t][:, :, 64:65], 1.0)
                    nc.vector.memset(va[t][:, :, 64:65], 1.0)
                for t in range(1, 5):
                    dma(kbgT[t], kbgT_d.ap()[t])
                for i in range(10):
                    dma(wv[i], wv_d.ap()[i])
                wo = [const.tile([128, 1280], BF16, name=f"wo{j}", tag=f"wo{j}")
                      for j in range(5)]
                for j in range(5):
                    dma(wo[j], wo_d.ap()[j])

                qT = [const.tile([128, 1024], BF16, name=f"qT{g}", tag=f"qT{g}")
                      for g in range(5)]
                kT = [const.tile([128, 1024], BF16, name=f"kT{g}", tag=f"kT{g}")
                      for g in range(5)]
                ctxn = [const.tile([128, 1024], BF16, name=f"ctxn{g}", tag=f"ctxn{g}")
                        for g in range(5)]

                # ---- projection emitters (used upfront for g=0, as PE
                # filler interleaved into attention for g>=1) ----
                def qk_group_steps(g, w_sb, dst, half, psum_tag):
                    """Two emission steps (5 matmuls each) accumulating one
                    [128, 512] q^T/k^T projection tile, then evacuate."""
                    gs = bass.ts(g, 128)
                    qs = bass.ts(half, 512)
                    state = {}

                    def step1():
                        state["ps"] = psum.tile(
                            [128, 512], F32, name=f"pp{g}{half}",
                            uniquify=True, tag=psum_tag,
                            bufs=2 if psum_tag == "sc" else 1)
                        for i in range(5):
                            nc.tensor.matmul(state["ps"], w_sb[i][:, gs],
                                             hT[i][:, qs],
                                             start=(i == 0), stop=False)

                    def step2():
                        for i in range(5, 10):
                            nc.tensor.matmul(state["ps"], w_sb[i][:, gs],
                                             hT[i][:, qs],
                                             start=False, stop=(i == 9))
                        nc.vector.tensor_copy(out=dst[g][:, qs], in_=state["ps"])

                    return [step1, step2]

                def v_group_step(g, t):
                    """One emission step: v projection for heads (2g, 2g+1),
                    token tile t -> va[t][:, 2g:2g+2, 0:64]."""
                    gs = bass.ts(g, 128)
                    ts_ = bass.ts(t, 128)

                    def step():
                        ps = psum.tile([128, 128], F32, name=f"pv{g}{t}",
                                       uniquify=True, tag="pp", bufs=1)
                        for i in range(10):
                            nc.tensor.matmul(ps, hT[i][:, ts_], wv[i][:, gs],
                                             start=(i == 0), stop=(i == 9))
                        nc.vector.tensor_copy(
                            out=va[t][:, 2 * g:2 * g + 2, 0:64],
                            in_=ps.rearrange("p (a b) -> p a b", a=2))

                    return step

                # g=0 half-0 q/k projections upfront (through the idle sc
                # slots); the half-1 projections ride as attention filler.
                for st in qk_group_steps(0, wq, qT, 0, "sc"):
                    st()
                for st in qk_group_steps(0, wk, kT, 0, "sc"):
                    st()

                CORDER = list(range(8, 16)) + list(range(8))  # bg chunks first

                def emit_qk_mm(g, c, half, sc):
                    """Row-packed scores^T matmul pair for kv-chunk c into
                    sc = [128, 1024] PSUM laid out [A | B]."""
                    if c < 8:
                        kt_src = kT[g][:, bass.ts(c, 128)]
                    else:
                        kt_src = kbgT[g][:, bass.ts(c - 8, 128)]
                    for p in range(2):
                        rows = slice(p * 64, p * 64 + 64)
                        nc.tensor.matmul(
                            sc[:, bass.ts(p, 512)],
                            kt_src[rows, :],
                            qT[g][rows, bass.ts(half, 512)],
                            start=True, stop=True,
                            tile_position=(p * 64, 0))

                for g in range(5):
                    for half in range(2):
                        qs = bass.ts(half, 512)
                        # filler steps for this (g, half)
                        filler = deque()
                        if half == 0:
                            if g == 0:
                                # kT columns are kv tokens: both halves are
                                # read by the self chunks below — emit the
                                # half-1 k projection first.
                                filler.extend(qk_group_steps(0, wk, kT, 1, "pp"))
                            for t in range(8):
                                filler.append(v_group_step(g, t))
                            if g == 0:
                                filler.extend(qk_group_steps(0, wq, qT, 1, "pp"))
                        elif g < 4:
                            for (w_sb, dst) in ((wq, qT), (wk, kT)):
                                for h2 in range(2):
                                    filler.extend(qk_group_steps(
                                        g + 1, w_sb, dst, h2, "pp"))

                        cps = [psum.tile([65, 512], F32, name=f"cx{g}{half}{p}",
                                         tag="cx", bufs=3) for p in range(2)]
                        sc_tiles = {}
                        c0 = CORDER[0]
                        sc_tiles[c0] = psum.tile([128, 1024], F32,
                                                 name=f"sc{g}{half}{c0}",
                                                 uniquify=True, tag="sc", bufs=2)
                        emit_qk_mm(g, c0, half, sc_tiles[c0])
                        for k, c in enumerate(CORDER):
                            e = work.tile([128, 1024], BF16, name=f"e{g}{half}{c}",
                                          uniquify=True, tag="e", bufs=3)
                            nc.scalar.activation(out=e, in_=sc_tiles.pop(c),
                                                 func=Exp, scale=SCALE)
                            if k + 1 < 16:
                                cn = CORDER[k + 1]
                                sc_tiles[cn] = psum.tile(
                                    [128, 1024], F32, name=f"sc{g}{half}{cn}",
                                    uniquify=True, tag="sc", bufs=2)
                                emit_qk_mm(g, cn, half, sc_tiles[cn])
                            # PE filler while ACT computes exp
                            if half == 0:
                                if (k < 8 or k % 2 == 0) and filler:
                                    filler.popleft()()
                            elif k % 2 == 0 and filler:
                                filler.popleft()()
                            for p in range(2):
                                v_src = (va if c < 8 else vb)[c % 8][:, 2 * g + p, :]
                                nc.tensor.matmul(
                                    cps[p], v_src, e[:, bass.ts(p, 512)],
                                    start=(k == 0), stop=(k == 15))
                        while filler:
                            filler.popleft()()

                        # ---- normalize context, stage as bf16 ctx^T ----
                        for p in range(2):
                            rows = slice(p * 64, p * 64 + 64)
                            cs = work.tile([65, 512], F32, name=f"cs{g}{half}{p}",
                                           uniquify=True, tag="cs", bufs=4)
                            nc.vector.tensor_copy(out=cs, in_=cps[p])
                            zp = work.tile([128, 4], F32, name=f"zp{g}{half}{p}",
                                           uniquify=True, tag="zp", bufs=4)
                            # partition-scatter the Z row directly SBUF->SBUF
                            nc.sync.dma_start(
                                out=zp,
                                in_=cs[64:65, :].rearrange(
                                    "p (a b) -> p a b", a=128))
                            rp = work.tile([128, 4], F32, name=f"rp{g}{half}{p}",
                                           uniquify=True, tag="rp", bufs=4)
                            nc.vector.reciprocal(rp, zp)
                            nc.sync.dma_start(
                                out=rd_d.ap()[g, p, half].rearrange(
                                    "(a b) -> a b", a=128),
                                in_=rp)
                            rflat = rd_d.ap()[g, p, half]
                            rb = work.tile([64, 512], F32, name=f"rb{g}{half}{p}",
                                           uniquify=True, tag="rb", bufs=4)
                            nc.gpsimd.dma_start(
                                out=rb,
                                in_=bass.AP(tensor=rflat.tensor,
                                            offset=rflat.offset,
                                            ap=[[0, 64]] + list(rflat.ap)))
                            nc.vector.tensor_mul(out=ctxn[g][rows, qs],
                                                 in0=cs[0:64, :], in1=rb)

                # ---- output projection: out^T = Wo_rows^T @ ctx^T ----
                for dt_ in range(10):
                    ds_ = bass.ts(dt_, 128)
                    for t in range(2):
                        ts_ = bass.ts(t, 512)
                        ps = psum.tile([128, 512], F32, name=f"po{dt_}{t}",
                                       uniquify=True, tag="cx", bufs=3)
                        for j in range(5):
                            nc.tensor.matmul(ps, wo[j][:, ds_], ctxn[j][:, ts_],
                                             start=(j == 0), stop=(j == 4))
                        osb = work.tile([128, 512], F32, name=f"o{dt_}{t}",
                                        uniquify=True, tag="osb", bufs=4)
                        nc.vector.tensor_copy(out=osb, in_=ps)
                        nc.sync.dma_start(
                            out=outT_d.ap()[dt_ * 128:(dt_ + 1) * 128, ts_],
                            in_=osb)

            if reps == 1:
                body()
            else:
                with tc.For_i(0, reps, 1) as it:
                    body(it)

    nc.compile()
    return nc


def shard_inputs(inputs):
    """Full fp32 inputs -> 8 per-core input maps (host-side cast/layout)."""
    hs = np.asarray(inputs["hidden_states"], dtype=np.float32)
    Wq = np.asarray(inputs["Wq"], dtype=np.float32)
    Wk = np.asarray(inputs["Wk"], dtype=np.float32)
    Wv = np.asarray(inputs["Wv"], dtype=np.float32)
    Wo = np.asarray(inputs["Wo"], dtype=np.float32)
    K_bg = np.asarray(inputs["K_bg"], dtype=np.float32)
    V_bg = np.asarray(inputs["V_bg"], dtype=np.float32)

    in_maps = []
    for c in range(N_CORES):
        b, half = divmod(c, 2)
        col0 = half * HPC * DH  # 0 or 640
        bh0 = b * H + half * HPC
        hT = np.ascontiguousarray(hs[b].T).astype(nbf16).reshape(10, 128, 1024)
        wq = np.ascontiguousarray(Wq[:, col0:col0 + 640]).astype(nbf16)
        wk = np.ascontiguousarray(Wk[:, col0:col0 + 640]).astype(nbf16)
        wv = np.ascontiguousarray(Wv[:, col0:col0 + 640]).astype(nbf16)
        wo = np.ascontiguousarray(Wo[col0:col0 + 640, :]).astype(nbf16)
        kbgT = np.ascontiguousarray(
            (ALPHA * K_bg[bh0:bh0 + 10]).transpose(0, 2, 1)).astype(nbf16)
        vbg = np.ascontiguousarray(
            (ALPHA * V_bg[bh0:bh0 + 10]).reshape(10, 8, 128, 64)
            .transpose(1, 2, 0, 3)).astype(nbf16)
        in_maps.append({
            "hT": hT,
            "wq": wq.reshape(10, 128, 640),
            "wk": wk.reshape(10, 128, 640),
            "wv": wv.reshape(10, 128, 640),
            "wo": wo.reshape(5, 128, 1280),
            "kbgT": kbgT.reshape(5, 128, 1024),
            "vbg": vbg,
        })
    return in_maps


_NC_CACHE = {}


def _get_nc(reps=1):
    if reps not in _NC_CACHE:
        _NC_CACHE[reps] = build_kernel(reps)
    return _NC_CACHE[reps]


def kernel(**inputs):
    nc = _get_nc(1)
    in_maps = shard_inputs(inputs)
    res = bass_utils.run_bass_kernel_spmd(nc, in_maps,
                                          core_ids=list(range(N_CORES)))
    outs = np.stack([res.results[c]["outT"] for c in range(N_CORES)])
    out = outs.reshape(B, 2, D, L).sum(axis=1).transpose(0, 2, 1)
    out = out + np.asarray(inputs["bo"], dtype=np.float32)
    return np.ascontiguousarray(out, dtype=np.float32)


# revision 14
# speedup vs baseline: 1.6211x; 1.2044x over previous
"""CARC attention processor kernel for 8 Trainium2 NeuronCores.

Reference computation (B=4, L=1024, D=1280, H=20, DH=64, ALPHA=0.48):
    q/k/v = hidden @ Wq/Wk/Wv               -> per-head [B*H, L, 64]
    k,v   = concat([k, a*K_bg], [v, a*V_bg]) along kv-seq (kt = 2L = 2048)
    out   = softmax(q k^T / 8) v  -> merge heads -> @ Wo + bo

Sharding: core = 2*b + half owns batch b and heads [half*10, half*10+10).
Each core computes its 10 (batch, head) pairs end-to-end and a partial
out^T = (context_heads^T) @ Wo_rows; host sums the two half-partials per
batch. ALPHA is folded into K_bg/V_bg on the host; all matmul operands
are bf16 (fp32 PSUM accumulation), softmax runs in fp32 on ACT.

Pipeline structure (per head-pair group g = heads 2g, 2g+1):
  - scores^T [kt=128, qt] per 128-chunk of kt, computed as a row-packed
    matmul pair (tile_position rows 0-63 / 64-127 = head A / head B) into
    one [128, 1024] PSUM tile laid out [A | B]; a single FD=1024 exp on
    ACT produces E (bf16).  PV accumulates ctx^T [65, qt] with a
    ones-augmented V (row 64 = softmax denominator Z).
  - The kv loop visits background chunks first so self-V projections
    (interleaved as PE filler) are ready when the self chunks arrive.
  - q/k projections for group g+1 are interleaved into group g's
    attention as PE filler, keeping ACT (the exp spine) saturated.
  - Z reciprocal: Z row -> DRAM -> [128,4] -> reciprocal -> DRAM ->
    partition-broadcast [64,512]; ctx^T * (1/Z) -> bf16 ctxn.
  - out^T accumulated from Wo chunks at the end.
"""

import sys

if "/opt/trn_rl_repo" not in sys.path:
    sys.path.insert(0, "/opt/trn_rl_repo")

from collections import deque
from contextlib import ExitStack

import numpy as np
import ml_dtypes

import concourse.bass as bass
import concourse.mybir as mybir
import concourse.tile as tile
from concourse import bacc
from concourse import bass_utils

B, L, D, H, DH = 4, 1024, 1280, 20, 64
ALPHA = 0.48
SCALE = 0.125  # 1/sqrt(DH)
HPC = 10       # heads per core
N_CORES = 8
BF16 = mybir.dt.bfloat16
F32 = mybir.dt.float32
nbf16 = ml_dtypes.bfloat16


import os
_PROBE = os.environ.get("KERNEL_PROBE", "")


def build_kernel(reps: int = 1):
    """Build + compile the per-core Bass module. reps>1 wraps the whole body
    in a hardware loop (used only for wall-clock timing in test harnesses)."""
    nc = bacc.Bacc("TRN2", target_bir_lowering=False, debug=False,
                   num_devices=N_CORES)

    hT_d = nc.dram_tensor("hT", [10, 128, 1024], BF16, kind="ExternalInput")
    wq_d = nc.dram_tensor("wq", [10, 128, 640], BF16, kind="ExternalInput")
    wk_d = nc.dram_tensor("wk", [10, 128, 640], BF16, kind="ExternalInput")
    wv_d = nc.dram_tensor("wv", [10, 128, 640], BF16, kind="ExternalInput")
    wo_d = nc.dram_tensor("wo", [5, 128, 1280], BF16, kind="ExternalInput")
    kbgT_d = nc.dram_tensor("kbgT", [5, 128, 1024], BF16, kind="ExternalInput")
    vbg_d = nc.dram_tensor("vbg", [8, 128, 10, 64], BF16, kind="ExternalInput")
    outT_d = nc.dram_tensor("outT", [1280, 1024], F32, kind="ExternalOutput")
    # scratch for the softmax-denominator reciprocal round trip
    zd_d = nc.dram_tensor("zd", [5, 2, 2, 512], F32, kind="Internal")
    rd_d = nc.dram_tensor("rd", [5, 2, 2, 512], F32, kind="Internal")

    with tile.TileContext(nc) as tc:
        with ExitStack() as ctx:
            const = ctx.enter_context(tc.tile_pool(name="const", bufs=1))
            work = ctx.enter_context(tc.tile_pool(name="work", bufs=1))
            psum = ctx.enter_context(tc.tile_pool(name="psum", bufs=1, space="PSUM"))

            def body(_it=None):
                Exp = mybir.ActivationFunctionType.Exp

                # ---- persistent SBUF tiles + input DMAs ----
                hT = [const.tile([128, 1024], BF16, name=f"hT{i}", tag=f"hT{i}")
                      for i in range(10)]
                wq = [const.tile([128, 640], BF16, name=f"wq{i}", tag=f"wq{i}")
                      for i in range(10)]
                wk = [const.tile([128, 640], BF16, name=f"wk{i}", tag=f"wk{i}")
                      for i in range(10)]
                # Interleave the startup loads across the SP HWDGE ring and
                # the gpsimd SWDGE ring (both idle at startup; the ACT ring
                # would delay the first activations behind its triggers).
                rings = [nc.sync, nc.gpsimd]
                _n = 0

                def dma(out, in_):
                    nonlocal _n
                    rings[_n % 2].dma_start(out=out, in_=in_)
                    _n += 1

                kbgT = [const.tile([128, 1024], BF16, name=f"kbgT{t}", tag=f"kbgT{t}")
                        for t in range(5)]
                dma(kbgT[0], kbgT_d.ap()[0])
                for i in range(10):
                    dma(hT[i], hT_d.ap()[i])
                    dma(wq[i], wq_d.ap()[i])
                    dma(wk[i], wk_d.ap()[i])
                va = [const.tile([128, 10, 65], BF16, name=f"va{t}", tag=f"va{t}")
                      for t in range(8)]
                vb = [const.tile([128, 10, 65], BF16, name=f"vb{t}", tag=f"vb{t}")
                      for t in range(8)]
                wv = [const.tile([128, 640], BF16, name=f"wv{i}", tag=f"wv{i}")
                      for i in range(10)]
                for t in range(8):
                    dma(vb[t][:, :, 0:64], vbg_d.ap()[t])
                    nc.vector.memset(vb[t][:, :, 64:65], 1.0)
                    nc.vector.memset(va[t][:, :, 64:65], 1.0)
                for t in range(1, 5):
                    dma(kbgT[t], kbgT_d.ap()[t])
                for i in range(10):
                    dma(wv[i], wv_d.ap()[i])
                wo = [const.tile([128, 1280], BF16, name=f"wo{j}", tag=f"wo{j}")
                      for j in range(5)]
                for j in range(5):
                    dma(wo[j], wo_d.ap()[j])

                qT = [const.tile([128, 1024], BF16, name=f"qT{g}", tag=f"qT{g}")
                      for g in range(5)]
                kT = [const.tile([128, 1024], BF16, name=f"kT{g}", tag=f"kT{g}")
                      for g in range(5)]
                ctxn = [const.tile([128, 1024], BF16, name=f"ctxn{g}", tag=f"ctxn{g}")
                        for g in range(5)]

                # ---- projection emitters (used upfront for g=0, as PE
                # filler interleaved into attention for g>=1) ----
                def qk_group_steps(g, w_sb, dst, half, psum_tag):
                    """Two emission steps (5 matmuls each) accumulating one
                    [128, 512] q^T/k^T projection tile, then evacuate."""
                    gs = bass.ts(g, 128)
                    qs = bass.ts(half, 512)
                    state = {}

                    r1 = range(5) if _PROBE != "noproj" else range(1)
                    r2 = range(5, 10) if _PROBE != "noproj" else range(9, 10)

                    def step1():
                        state["ps"] = psum.tile(
                            [128, 512], F32, name=f"pp{g}{half}",
                            uniquify=True, tag=psum_tag,
                            bufs=2 if psum_tag == "sc" else 1)
                        for i in r1:
                            nc.tensor.matmul(state["ps"], w_sb[i][:, gs],
                                             hT[i][:, qs],
                                             start=(i == r1[0]), stop=False)

                    def step2():
                        for i in r2:
                            nc.tensor.matmul(state["ps"], w_sb[i][:, gs],
                                             hT[i][:, qs],
                                             start=False, stop=(i == 9))
                        nc.vector.tensor_copy(out=dst[g][:, qs], in_=state["ps"])

                    return [step1, step2]

                def v_group_step(g, t):
                    """One emission step: v projection for heads (2g, 2g+1),
                    token tile t -> va[t][:, 2g:2g+2, 0:64]."""
                    gs = bass.ts(g, 128)
                    ts_ = bass.ts(t, 128)

                    rr = range(10) if _PROBE != "noproj" else range(9, 10)

                    def step():
                        ps = psum.tile([128, 128], F32, name=f"pv{g}{t}",
                                       uniquify=True, tag="pp", bufs=1)
                        for i in rr:
                            nc.tensor.matmul(ps, hT[i][:, ts_], wv[i][:, gs],
                                             start=(i == rr[0]), stop=(i == 9))
                        nc.vector.tensor_copy(
                            out=va[t][:, 2 * g:2 * g + 2, 0:64],
                            in_=ps.rearrange("p (a b) -> p a b", a=2))

                    return step

                # g=0 half-0 q/k projections upfront (through the idle sc
                # slots); the half-1 projections ride as attention filler.
                for st in qk_group_steps(0, wq, qT, 0, "sc"):
                    st()
                for st in qk_group_steps(0, wk, kT, 0, "sc"):
                    st()

                CORDER = list(range(8, 16)) + list(range(8))  # bg chunks first

                def emit_qk_mm(g, c, half, sc):
                    """Row-packed scores^T matmul pair for kv-chunk c into
                    sc = [128, 1024] PSUM laid out [A | B]."""
                    if c < 8:
                        kt_src = kT[g][:, bass.ts(c, 128)]
                    else:
                        kt_src = kbgT[g][:, bass.ts(c - 8, 128)]
                    for p in range(2):
                        rows = slice(p * 64, p * 64 + 64)
                        nc.tensor.matmul(
                            sc[:, bass.ts(p, 512)],
                            kt_src[rows, :],
                            qT[g][rows, bass.ts(half, 512)],
                            start=True, stop=True,
                            tile_position=(p * 64, 0))

                for g in range(5):
                    for half in range(2):
                        qs = bass.ts(half, 512)
                        # filler steps for this (g, half)
                        filler = deque()
                        if half == 0:
                            if g == 0:
                                # kT columns are kv tokens: both halves are
                                # read by the self chunks below — emit the
                                # half-1 k projection first.
                                filler.extend(qk_group_steps(0, wk, kT, 1, "pp"))
                            for t in range(8):
                                filler.append(v_group_step(g, t))
                            if g == 0:
                                filler.extend(qk_group_steps(0, wq, qT, 1, "pp"))
                        elif g < 4:
                            for (w_sb, dst) in ((wq, qT), (wk, kT)):
                                for h2 in range(2):
                                    filler.extend(qk_group_steps(
                                        g + 1, w_sb, dst, h2, "pp"))

                        cps = [psum.tile([65, 512], F32, name=f"cx{g}{half}{p}",
                                         tag="cx", bufs=3) for p in range(2)]
                        sc_tiles = {}
                        c0 = CORDER[0]
                        sc_tiles[c0] = psum.tile([128, 1024], F32,
                                                 name=f"sc{g}{half}{c0}",
                                                 uniquify=True, tag="sc", bufs=2)
                        emit_qk_mm(g, c0, half, sc_tiles[c0])
                        for k, c in enumerate(CORDER):
                            e = work.tile([128, 1024], BF16, name=f"e{g}{half}{c}",
                                          uniquify=True, tag="e", bufs=3)
                            sct = sc_tiles.pop(c)
                            if _PROBE == "smallexp":  # timing probe only
                                nc.scalar.activation(out=e[:, 0:64],
                                                     in_=sct[:, 0:64],
                                                     func=Exp, scale=SCALE)
                            else:
                                nc.scalar.activation(out=e, in_=sct,
                                                     func=Exp, scale=SCALE)
                            if k + 1 < 16:
                                cn = CORDER[k + 1]
                                sc_tiles[cn] = psum.tile(
                                    [128, 1024], F32, name=f"sc{g}{half}{cn}",
                                    uniquify=True, tag="sc", bufs=2)
                                emit_qk_mm(g, cn, half, sc_tiles[cn])
                            # PE filler while ACT computes exp
                            if half == 0:
                                if (k < 8 or k % 2 == 0) and filler:
                                    filler.popleft()()
                            elif k % 2 == 0 and filler:
                                filler.popleft()()
                            for p in range(2):
                                v_src = (va if c < 8 else vb)[c % 8][:, 2 * g + p, :]
                                nc.tensor.matmul(
                                    cps[p], v_src, e[:, bass.ts(p, 512)],
                                    start=(k == 0), stop=(k == 15))
                        while filler:
                            filler.popleft()()

                        # ---- normalize context, stage as bf16 ctx^T ----
                        for p in range(2):
                            rows = slice(p * 64, p * 64 + 64)
                            cs = work.tile([65, 512], F32, name=f"cs{g}{half}{p}",
                                           uniquify=True, tag="cs", bufs=4)
                            nc.vector.tensor_copy(out=cs, in_=cps[p])
                            zp = work.tile([128, 4], F32, name=f"zp{g}{half}{p}",
                                           uniquify=True, tag="zp", bufs=4)
                            # partition-scatter the Z row directly SBUF->SBUF
                            nc.sync.dma_start(
                                out=zp,
                                in_=cs[64:65, :].rearrange(
                                    "p (a b) -> p a b", a=128))
                            rp = work.tile([128, 4], F32, name=f"rp{g}{half}{p}",
                                           uniquify=True, tag="rp", bufs=4)
                            nc.vector.reciprocal(rp, zp)
                            nc.sync.dma_start(
                                out=rd_d.ap()[g, p, half].rearrange(
                                    "(a b) -> a b", a=128),
                                in_=rp)
                            rflat = rd_d.ap()[g, p, half]
                            rb = work.tile([64, 512], F32, name=f"rb{g}{half}{p}",
                                           uniquify=True, tag="rb", bufs=4)
                            nc.gpsimd.dma_start(
                                out=rb,
                                in_=bass.AP(tensor=rflat.tensor,
                                            offset=rflat.offset,
                                            ap=[[0, 64]] + list(rflat.ap)))
                            nc.vector.tensor_mul(out=ctxn[g][rows, qs],
                                                 in0=cs[0:64, :], in1=rb)

                # ---- output projection: out^T = Wo_rows^T @ ctx^T ----
                for dt_ in range(10):
                    ds_ = bass.ts(dt_, 128)
                    for t in range(2):
                        ts_ = bass.ts(t, 512)
                        ps = psum.tile([128, 512], F32, name=f"po{dt_}{t}",
                                       uniquify=True, tag="cx", bufs=3)
                        for j in range(5):
                            nc.tensor.matmul(ps, wo[j][:, ds_], ctxn[j][:, ts_],
                                             start=(j == 0), stop=(j == 4))
                        osb = work.tile([128, 512], F32, name=f"o{dt_}{t}",
                                        uniquify=True, tag="osb", bufs=4)
                        nc.vector.tensor_copy(out=osb, in_=ps)
                        nc.sync.dma_start(
                            out=outT_d.ap()[dt_ * 128:(dt_ + 1) * 128, ts_],
                            in_=osb)

            if reps == 1:
                body()
            else:
                with tc.For_i(0, reps, 1) as it:
                    body(it)

    nc.compile()
    return nc


def shard_inputs(inputs):
    """Full fp32 inputs -> 8 per-core input maps (host-side cast/layout)."""
    hs = np.asarray(inputs["hidden_states"], dtype=np.float32)
    Wq = np.asarray(inputs["Wq"], dtype=np.float32)
    Wk = np.asarray(inputs["Wk"], dtype=np.float32)
    Wv = np.asarray(inputs["Wv"], dtype=np.float32)
    Wo = np.asarray(inputs["Wo"], dtype=np.float32)
    K_bg = np.asarray(inputs["K_bg"], dtype=np.float32)
    V_bg = np.asarray(inputs["V_bg"], dtype=np.float32)

    in_maps = []
    for c in range(N_CORES):
        b, half = divmod(c, 2)
        col0 = half * HPC * DH  # 0 or 640
        bh0 = b * H + half * HPC
        hT = np.ascontiguousarray(hs[b].T).astype(nbf16).reshape(10, 128, 1024)
        wq = np.ascontiguousarray(Wq[:, col0:col0 + 640]).astype(nbf16)
        wk = np.ascontiguousarray(Wk[:, col0:col0 + 640]).astype(nbf16)
        wv = np.ascontiguousarray(Wv[:, col0:col0 + 640]).astype(nbf16)
        wo = np.ascontiguousarray(Wo[col0:col0 + 640, :]).astype(nbf16)
        kbgT = np.ascontiguousarray(
            (ALPHA * K_bg[bh0:bh0 + 10]).transpose(0, 2, 1)).astype(nbf16)
        vbg = np.ascontiguousarray(
            (ALPHA * V_bg[bh0:bh0 + 10]).reshape(10, 8, 128, 64)
            .transpose(1, 2, 0, 3)).astype(nbf16)
        in_maps.append({
            "hT": hT,
            "wq": wq.reshape(10, 128, 640),
            "wk": wk.reshape(10, 128, 640),
            "wv": wv.reshape(10, 128, 640),
            "wo": wo.reshape(5, 128, 1280),
            "kbgT": kbgT.reshape(5, 128, 1024),
            "vbg": vbg,
        })
    return in_maps


_NC_CACHE = {}


def _get_nc(reps=1):
    if reps not in _NC_CACHE:
        _NC_CACHE[reps] = build_kernel(reps)
    return _NC_CACHE[reps]


def kernel(**inputs):
    nc = _get_nc(1)
    in_maps = shard_inputs(inputs)
    res = bass_utils.run_bass_kernel_spmd(nc, in_maps,
                                          core_ids=list(range(N_CORES)))
    outs = np.stack([res.results[c]["outT"] for c in range(N_CORES)])
    out = outs.reshape(B, 2, D, L).sum(axis=1).transpose(0, 2, 1)
    out = out + np.asarray(inputs["bo"], dtype=np.float32)
    return np.ascontiguousarray(out, dtype=np.float32)
